# revision 7
# baseline (speedup 1.0000x reference)
"""Trainium2 Bass kernel for nn_GammaSpaceLayer.

SSM with fixed "gamma" transition A (bidiagonal), bilinear discretization,
kernel k[l] = C dA^l dB, FFT causal conv + D*u skip.  Computed as a chunked
linear scan (state dim H=64, chunk T=8):

  z_t[c]   = sum_{s<=t} dA^{t-s} dB u[c,s]          (intra, pair-packed)
  x_end[c] = A8 x_end[c-1] + z_7[c]                 (radix-4 log scan)
  y_t[c]   = C z_t[c] + C dA^{t+1} x_end[c-1] + D u_t[c]

All matmul operands are bf16 (1 cy/row on the PE at any N); accumulation in
fp32 PSUM.  Layout work (transposes to channel-major, t-major column order)
is done on the host; small input-dependent matrices (powers of dA etc.) are
precomputed on host in float64 and passed as inputs, so the Bass program is
input-independent.  Data-parallel over batch: 16 batches over 8 cores.
"""

import numpy as np
import ml_dtypes

import concourse.bass as bass
import concourse.mybir as mybir
import concourse.tile as tile
from concourse.vector_clock import ScopedClock
from concourse.bass_utils import run_bass_kernel_spmd

# problem constants (hardcoded per contract)
H, S = 64, 128          # state dim, io channel dim
B, L = 16, 2048         # full batch, seq len
N_CORES = 8
PB = B // N_CORES       # batches per core (2)
T = 8                   # chunk length
CK = L // T             # chunks per batch (256)
COLS = PB * CK          # state columns per core (512)
NTOK = PB * L           # token columns per core (4096)
DT_MIN, DT_MAX = 0.001, 0.1

F32 = mybir.dt.float32
BF = mybir.dt.bfloat16
BF_NP = ml_dtypes.bfloat16

# weight layout offsets
#  w128 (128 rows): [ GTX: 9 blocks of 64 (zero, G0..G7, transposed) | DD: 128 ]
GTX_OFF = 0
DD_OFF = 9 * H          # 576
W128_COLS = DD_OFF + S  # 704
#  w64 (64 rows): [ CAT: 8 blocks of 128 | SW: 12 blocks of 64 | CT: 128 ]
CAT_OFF = 0
SW_OFF = T * S          # 1024
CT_OFF = SW_OFF + 12 * H  # 1792
W64_COLS = CT_OFF + S   # 1920

N_WARM = 6              # PE warm-up matmuls (ramp p-state during input DMA)


class _TC(tile.TileContext):
    """TileContext whose tail drain splits multi-sem waits: this walrus
    build caps CTRL instructions at one sync-wait command."""

    def _drain_and_barrier(self, tick_clock, wait_clock):
        probe = self.nc.sync.drain()
        wait_clock.add_sem_waits(probe.ins, ScopedClock({None: tick_clock.global_clock}))
        si = probe.ins.sync_info
        if si is not None and si.on_wait and len(si.on_wait) > 1:
            waits = list(si.on_wait)
            probe.ins.sync_info = mybir.SyncInfo(
                on_wait=[waits[0]], on_update=list(si.on_update or []))
            for w in waits[1:]:
                d = self.nc.sync.drain()
                d.ins.sync_info = mybir.SyncInfo(on_wait=[w], on_update=[])
        self.nc.all_engine_barrier()
        assert self.sems is not None
        popped = self.nc._tile_sem_poison_stack.pop()
        assert popped is self._sem_poison
        self.nc.clear_and_free_semaphores(list(self.sems.allocated().values()))
        self.nc.all_engine_barrier()


def _split_multi_waits(nc):
    """This walrus build allows only ONE sync-wait command per instruction.
    Split extras onto same-engine InstEventSemaphore carriers inserted
    immediately before (engine program order preserves semantics)."""
    n = 0
    for f in nc.m.functions:
        for b in f.blocks:
            il = b.instructions
            i = 0
            while i < len(il):
                ins = il[i]
                si = ins.sync_info
                if si is not None and si.on_wait and len(si.on_wait) > 1:
                    waits = list(si.on_wait)
                    ins.sync_info = mybir.SyncInfo(
                        on_wait=[waits[-1]], on_update=list(si.on_update or []))
                    for j, w in enumerate(waits[:-1]):
                        ev = mybir.InstEventSemaphore(
                            name=f"{ins.name}_wsplit{j}", ins=[], outs=[])
                        ev.engine = ins.engine
                        ev.sync_info = mybir.SyncInfo(on_wait=[w], on_update=[])
                        il.insert(i, ev)
                        i += 1
                        n += 1
                i += 1
    return n


def _build():
    nc = bass.Bass()
    u_d = nc.dram_tensor("u", [S, NTOK], BF, kind="ExternalInput")      # (i, t b c)
    w128_d = nc.dram_tensor("W128", [S, W128_COLS], BF, kind="ExternalInput")
    w64_d = nc.dram_tensor("W64", [H, W64_COLS], BF, kind="ExternalInput")
    y_d = nc.dram_tensor("y", [S, NTOK], BF, kind="ExternalOutput")     # (o, t b c)

    with _TC(nc) as tc:
        with (
            tc.tile_pool(name="const", bufs=1) as cpool,
            tc.tile_pool(name="work", bufs=3, space="PSUM") as wpool,
            tc.tile_pool(name="ypsum", bufs=5, space="PSUM") as ypool,
        ):
            # ---- SBUF tiles ----
            u_sb = cpool.tile([S, NTOK], BF)
            w128 = cpool.tile([S, W128_COLS], BF)
            w64 = cpool.tile([H, W64_COLS], BF)
            wsrc = cpool.tile([S, COLS], BF)          # warm-up source (zeros)
            sc_a = cpool.tile([H, PB * 2 * CK], BF)   # scan ping
            sc_b = cpool.tile([H, PB * 2 * CK], BF)   # scan pong
            z_sb = cpool.tile([H, T * COLS], BF)      # intra states, per t
            y_sb = cpool.tile([S, NTOK], BF)          # output staging

            sa = sc_a[:].rearrange("p (b x) -> p b x", b=PB)
            sb = sc_b[:].rearrange("p (b x) -> p b x", b=PB)

            def useg(t):
                return u_sb[:, t * COLS:(t + 1) * COLS]

            def zseg(t):
                return z_sb[:, t * COLS:(t + 1) * COLS]

            def yseg(t):
                return y_sb[:, t * COLS:(t + 1) * COLS]

            # ---- memsets (gpsimd) ----
            nc.gpsimd.memset(wsrc[:], 0)
            nc.gpsimd.memset(sa[:, :, 0:CK], 0)
            nc.gpsimd.memset(sb[:, :, 0:CK], 0)

            # ---- DMAs (SP queue): weights for intra first, then u slices ----
            nc.sync.dma_start(w128[:], w128_d[:])
            for p in range(4):
                sl = slice(p * 2 * COLS, (p + 1) * 2 * COLS)
                nc.sync.dma_start(u_sb[:, sl], u_d[:, sl])
            nc.sync.dma_start(w64[:], w64_d[:])

            # ---- PE warm-up: ramp p-state while DMAs land ----
            warm = wpool.tile([S, COLS], F32, tag="w")
            for i in range(N_WARM):
                nc.tensor.matmul(warm[:], wsrc[:, 0:S], wsrc[:],
                                 start=(i == 0), stop=(i == N_WARM - 1))
            nc.scalar.copy(y_sb[0:1, 0:1], warm[0:1, 0:1])  # keep tile "read"

            # ---- intra-chunk states, pair-packed: pair q holds (z_2q; z_2q+1)
            # lhsT for (q, s) = w128[:, 64*(2q-s+1) : 64*(2q-s+1)+128]
            # (contiguous [G_{2q-s} | G_{2q+1-s}] in the GTX layout).
            qt = {}

            def intra(q, s, start, stop):
                nc.tensor.matmul(
                    qt[q][:], w128[:, H * (2 * q - s + 1): H * (2 * q - s + 1) + S],
                    useg(s), start=start, stop=stop)

            # issue order interleaves pair 3 (the scan seed) first, chasing
            # the u slice DMAs, with the other pairs as fill.
            qt[3] = wpool.tile([S, COLS], F32, tag="w", name="q3")
            qt[0] = wpool.tile([S, COLS], F32, tag="w", name="q0")
            intra(3, 0, True, False)
            intra(3, 1, False, False)
            intra(0, 0, True, False)
            intra(0, 1, False, True)
            # pair 0 done -> stage z_0, z_1
            nc.scalar.copy(zseg(0), qt[0][0:H, :])
            nc.vector.tensor_copy(zseg(1), qt[0][H:S, :])
            intra(3, 2, False, False)
            intra(3, 3, False, False)
            qt[1] = wpool.tile([S, COLS], F32, tag="w", name="q1")
            intra(1, 0, True, False)
            intra(1, 1, False, False)
            intra(3, 4, False, False)
            intra(3, 5, False, False)
            intra(1, 2, False, False)
            intra(1, 3, False, True)
            nc.scalar.copy(zseg(2), qt[1][0:H, :])
            nc.vector.tensor_copy(zseg(3), qt[1][H:S, :])
            intra(3, 6, False, False)
            intra(3, 7, False, True)
            # pair 3 done -> scan seed + z_6, z_7
            nc.vector.tensor_copy(sa[:, :, CK:2 * CK],
                                  qt[3][H:S, :].rearrange("p (b c) -> p b c", b=PB))
            nc.scalar.copy(zseg(7), qt[3][H:S, :])
            nc.scalar.copy(zseg(6), qt[3][0:H, :])

            # ---- y accumulators: D*u first (no scan dependency) ----
            yt = {}

            def dmm(t):
                yt[t] = ypool.tile([S, COLS], F32, tag="y", name=f"yt{t}")
                nc.tensor.matmul(yt[t][:], w128[:, DD_OFF:DD_OFF + S], useg(t),
                                 start=True, stop=False)

            def cmm(t):
                nc.tensor.matmul(yt[t][:], w64[:, CT_OFF:CT_OFF + S], zseg(t),
                                 start=False, stop=False)

            def camm(t):
                nc.tensor.matmul(yt[t][:], w64[:, S * t:S * (t + 1)],
                                 cur[:, :, CK - 1:2 * CK - 1], start=False, stop=True)

            dmm(0)
            dmm(1)
            dmm(2)

            # pair 2 (fill during early scan)
            qt[2] = wpool.tile([S, COLS], F32, tag="w", name="q2")
            intra(2, 0, True, False)
            for s in range(1, 6):
                intra(2, s, False, s == 5)
            nc.scalar.copy(zseg(4), qt[2][0:H, :])
            nc.vector.tensor_copy(zseg(5), qt[2][H:S, :])

            dmm(3)
            dmm(4)

            # ---- chunk-state scan: radix-4 Hillis-Steele, 4 levels ----
            cur, nxt = sa, sb
            for d in range(4):
                step = 4 ** d
                ps = wpool.tile([S, COLS], F32, tag="w", name=f"scan{d}")
                for k in (1, 2, 3):
                    sh = k * step
                    w = SW_OFF + H * (3 * d + (k - 1))
                    nc.tensor.matmul(ps[0:H, :], w64[:, w:w + H],
                                     cur[:, :, CK - sh:2 * CK - sh],
                                     start=(k == 1), stop=(k == 3))
                # PE fill between scan levels: C-applies for early t
                if d == 0:
                    cmm(0)
                    cmm(1)
                elif d == 1:
                    cmm(2)
                    cmm(3)
                elif d == 2:
                    cmm(4)
                nc.vector.tensor_add(nxt[:, :, CK:2 * CK],
                                     ps[0:H, :].rearrange("p (b c) -> p b c", b=PB),
                                     cur[:, :, CK:2 * CK])
                cur, nxt = nxt, cur
            # cur = full prefix states x_end[c]; xprev = shift right by 1 chunk

            # ---- finish y: + C dA^{t+1} x_end[c-1], then t=5..7 groups ----
            for t in range(5):
                camm(t)
            eng = [nc.vector.tensor_copy, nc.scalar.copy]
            for t in range(5):
                eng[t % 2](yseg(t), yt[t][:])
                nc.sync.dma_start(y_d[:, t * COLS:(t + 1) * COLS], yseg(t))
            for t in (5, 6, 7):
                yt[t] = wpool.tile([S, COLS], F32, tag="w", name=f"ytl{t}")
                nc.tensor.matmul(yt[t][:], w128[:, DD_OFF:DD_OFF + S], useg(t),
                                 start=True, stop=False)
                cmm(t)
                camm(t)
                eng[t % 2](yseg(t), yt[t][:])
                nc.sync.dma_start(y_d[:, t * COLS:(t + 1) * COLS], yseg(t))

    _split_multi_waits(nc)
    return nc


_NC_CACHE = {}


def _get_nc():
    if "nc" not in _NC_CACHE:
        _NC_CACHE["nc"] = _build()
    return _NC_CACHE["nc"]


def _host_precompute(Bmat, Cmat, Dvec, log_dt):
    Bm = np.asarray(Bmat, dtype=np.float64)
    Cm = np.asarray(Cmat, dtype=np.float64)
    Dv = np.asarray(Dvec, dtype=np.float64)
    x = np.float64(log_dt)
    dt = np.clip(np.logaddexp(0.0, x), DT_MIN, DT_MAX)   # softplus, clipped
    A = -np.eye(H) + np.eye(H, k=-1)
    back = np.eye(H) - 0.5 * dt * A
    fwd = np.eye(H) + 0.5 * dt * A
    dA = np.linalg.solve(back, fwd)
    dB = np.linalg.solve(back, dt * Bm)                  # (H, S)

    G = [dB]
    for _ in range(1, T):
        G.append(dA @ G[-1])
    A8 = np.linalg.matrix_power(dA, T)

    w128 = np.zeros((S, W128_COLS), dtype=np.float64)
    for d in range(T):                                   # GTX blocks 1..8
        w128[:, H * (d + 1): H * (d + 2)] = G[d].T
    w128[:, DD_OFF:DD_OFF + S] = np.diag(Dv)

    w64 = np.zeros((H, W64_COLS), dtype=np.float64)
    dApow = dA.copy()                                    # dA^{t+1}
    for t in range(T):
        if t:
            dApow = dApow @ dA
        w64[:, S * t:S * (t + 1)] = (Cm @ dApow).T       # CAT blocks
    for d in range(4):                                   # SW blocks
        for k in (1, 2, 3):
            w64[:, SW_OFF + H * (3 * d + k - 1):
                 SW_OFF + H * (3 * d + k)] = np.linalg.matrix_power(A8, k * 4 ** d).T
    w64[:, CT_OFF:CT_OFF + S] = Cm.T

    return {
        "W128": np.ascontiguousarray(w128, dtype=BF_NP),
        "W64": np.ascontiguousarray(w64, dtype=BF_NP),
    }


def kernel(u, B, C, D, log_dt, _trace=False):
    pre = _host_precompute(B, C, D, log_dt)
    nc = _get_nc()

    u = np.asarray(u, dtype=np.float32)
    in_maps = []
    for k in range(N_CORES):
        uc = u[k * PB:(k + 1) * PB]                       # (PB, L, S)
        # (b, c, t, i) -> (i, t, b, c) -> (S, NTOK)
        ut = uc.reshape(PB, CK, T, S).transpose(3, 2, 0, 1).reshape(S, NTOK)
        in_maps.append({"u": np.ascontiguousarray(ut).astype(BF_NP), **pre})

    res = run_bass_kernel_spmd(nc, in_maps, core_ids=list(range(N_CORES)),
                               trace=_trace)
    outs = []
    for k in range(N_CORES):
        yk = np.asarray(res.results[k]["y"]).astype(np.float32)  # (S, NTOK)
        # (o, t, b, c) -> (b, c, t, o) -> (PB, L, S)
        yk = yk.reshape(S, T, PB, CK).transpose(2, 3, 1, 0).reshape(PB, L, S)
        outs.append(yk)
    y = np.concatenate(outs, axis=0)
    if _trace:
        kernel.last_result = res
    return y
